# revision 3
# baseline (speedup 1.0000x reference)
"""Trainium2 Bass kernel for the ASAP dual-branch GNN (GraphConv mean-aggr).

Strategy (data-parallel over graphs, 32 graphs per NeuronCore):
  * Host folds each graph's edge list into a dense normalized adjacency
    An[src, dst] = count(src->dst) / max(deg_dst, 1) in bf16 (one bincount
    over all edges), so on-device the GraphConv layers are pure dense
    matmuls: h = relu( wrel^T (x An) + wroot^T x + brel ).
  * An is split into two 100-row src halves so it can serve as the PE
    moving operand with the node-major ys = x@wrel halves as weights.
  * relu+bias on ACT with the per-graph pooled readout taken for free via
    activation accum_out.
  * Small MLP head + log_softmax computed per-core in f32; no collectives.
Host side only does sharding/layout: adjacency histogram + degree fold,
transposes, dtype casts, and constant folding of the mean-pool 1/200 into
lin1_w.
"""

import os
import sys

import numpy as np

if "/opt/trn_rl_repo" not in sys.path:
    sys.path.insert(0, "/opt/trn_rl_repo")

B, N, EPG = 256, 200, 3200
F, H, C = 200, 128, 2
NCORES = 8
GPC = B // NCORES  # graphs per core
NQ = 100  # src-half width

_CACHE = {}


def _f32(x):
    return np.ascontiguousarray(x, dtype=np.float32)


def _bf16(x):
    import ml_dtypes

    return np.ascontiguousarray(np.asarray(x, dtype=np.float32).astype(ml_dtypes.bfloat16))


def _build(gpc=GPC, repeat=1):
    import concourse.bass as bass
    import concourse.tile as tile
    from concourse import bacc, mybir

    dt = mybir.dt
    AF = mybir.ActivationFunctionType
    OP = mybir.AluOpType

    nc = bacc.Bacc("TRN2", target_bir_lowering=False, debug=False)

    # ---- DRAM I/O ----
    xt_d = nc.dram_tensor("xt", [2, gpc, F, N], dt.bfloat16, kind="ExternalInput").ap()
    an_d = nc.dram_tensor("an", [2, gpc, NQ, 2, N], dt.bfloat16, kind="ExternalInput").ap()
    wa_d = nc.dram_tensor("wa", [2, 2, 128, H], dt.bfloat16, kind="ExternalInput").ap()
    wb_d = nc.dram_tensor("wb", [2, 2, F - 128, H], dt.bfloat16, kind="ExternalInput").ap()
    w2_d = nc.dram_tensor("w2", [2, 2, H, H], dt.bfloat16, kind="ExternalInput").ap()
    br_d = nc.dram_tensor("brel", [2, 2, H, 1], dt.float32, kind="ExternalInput").ap()
    l1w_d = nc.dram_tensor("l1w", [4, H, H], dt.float32, kind="ExternalInput").ap()
    l1b_d = nc.dram_tensor("l1b", [1, H], dt.float32, kind="ExternalInput").ap()
    l2w_d = nc.dram_tensor("l2w", [H, H // 2], dt.float32, kind="ExternalInput").ap()
    l2b_d = nc.dram_tensor("l2b", [1, H // 2], dt.float32, kind="ExternalInput").ap()
    l3w_d = nc.dram_tensor("l3w", [H // 2, C], dt.float32, kind="ExternalInput").ap()
    l3b_d = nc.dram_tensor("l3b", [1, C], dt.float32, kind="ExternalInput").ap()
    out_d = nc.dram_tensor("out", [gpc, C], dt.float32, kind="ExternalOutput").ap()

    # ---- inline constants ----
    onesrow_f_d = nc.inline_tensor(_f32(np.ones((1, 128), np.float32)), "onesrowf").ap()
    ident_d = nc.inline_tensor(_f32(np.eye(128, dtype=np.float32)), "identf").ap()

    with tile.TileContext(nc) as tc:
        with (
            tc.tile_pool(name="cpool", bufs=1) as cpool,
            tc.tile_pool(name="xpool", bufs=6) as xpool,
            tc.tile_pool(name="apool", bufs=6) as apool,
            tc.tile_pool(name="hpool", bufs=6) as hpool,
            tc.tile_pool(name="rpool", bufs=4) as rpool,
            tc.tile_pool(name="psC", bufs=2, space="PSUM") as psC,
            tc.tile_pool(name="psD", bufs=2, space="PSUM") as psD,
        ):
            # ---- load constants / weights ----
            onesrow_f = cpool.tile([1, 128], dt.float32)
            nc.sync.dma_start(out=onesrow_f[:], in_=onesrow_f_d[:])
            ident = cpool.tile([128, 128], dt.float32)
            nc.sync.dma_start(out=ident[:], in_=ident_d[:])

            wa = cpool.tile([128, 2, 2, H], dt.bfloat16)
            wb = cpool.tile([F - 128, 2, 2, H], dt.bfloat16)
            w2 = cpool.tile([128, 2, 2, H], dt.bfloat16)
            brl = cpool.tile([128, 2, 2, 1], dt.float32)
            for b in range(2):
                for k in range(2):
                    nc.sync.dma_start(out=wa[:, b, k, :], in_=wa_d[b, k])
                    nc.sync.dma_start(out=wb[:, b, k, :], in_=wb_d[b, k])
                    nc.sync.dma_start(out=w2[:, b, k, :], in_=w2_d[b, k])
                    nc.sync.dma_start(out=brl[:, b, k, :], in_=br_d[b, k])
            l1w = cpool.tile([128, 4, H], dt.float32)
            for k in range(4):
                nc.sync.dma_start(out=l1w[:, k, :], in_=l1w_d[k])
            l1b = cpool.tile([1, H], dt.float32)
            nc.sync.dma_start(out=l1b[:], in_=l1b_d[:])
            l2w = cpool.tile([H, H // 2], dt.float32)
            nc.sync.dma_start(out=l2w[:], in_=l2w_d[:])
            l2b = cpool.tile([1, H // 2], dt.float32)
            nc.sync.dma_start(out=l2b[:], in_=l2b_d[:])
            l3w = cpool.tile([H // 2, C], dt.float32)
            nc.sync.dma_start(out=l3w[:], in_=l3w_d[:])
            l3b = cpool.tile([1, C], dt.float32)
            nc.sync.dma_start(out=l3b[:], in_=l3b_d[:])

            pooled = [
                [cpool.tile([128, gpc], dt.float32, tag=f"pool{b}{l}", name=f"pooled{b}{l}") for l in range(2)]
                for b in range(2)
            ]

            def conv_layers(b, g, A, xta, xtb):
                # Layer 1: ys = x @ wrel (node-major halves)
                yr = psC.tile([NQ, 2, H], dt.float32, tag="yr", name=f"yr{b}{g}")
                for m in range(2):
                    nc.tensor.matmul(
                        yr[:, m, :], lhsT=xta[:, m * NQ : (m + 1) * NQ], rhs=wa[:, b, 0, :],
                        start=True, stop=False,
                    )
                    nc.tensor.matmul(
                        yr[:, m, :], lhsT=xtb[:, m * NQ : (m + 1) * NQ], rhs=wb[:, b, 0, :],
                        start=False, stop=True,
                    )
                ys0 = hpool.tile([NQ, H], dt.bfloat16, tag="ys0", name=f"ys0{b}{g}")
                nc.scalar.copy(ys0[:], yr[:, 0, :])
                ys1 = hpool.tile([NQ, H], dt.bfloat16, tag="ys1", name=f"ys1{b}{g}")
                nc.scalar.copy(ys1[:], yr[:, 1, :])

                hp = psD.tile([128, N], dt.float32, tag="hD", name=f"hp{b}{g}")
                nc.tensor.matmul(hp[:], lhsT=wa[:, b, 1, :], rhs=xta[:], start=True, stop=False)
                nc.tensor.matmul(hp[:], lhsT=wb[:, b, 1, :], rhs=xtb[:], start=False, stop=False)
                nc.tensor.matmul(hp[:], lhsT=ys0[:], rhs=A[:, 0, :], start=False, stop=False)
                nc.tensor.matmul(hp[:], lhsT=ys1[:], rhs=A[:, 1, :], start=False, stop=True)
                h1 = hpool.tile([128, N], dt.bfloat16, tag="h1", name=f"h1{b}{g}")
                nc.scalar.activation(
                    h1[:], hp[:], AF.Relu, bias=brl[:, b, 0, :], scale=1.0,
                    accum_out=pooled[b][0][:, g : g + 1],
                )

                # Layer 2
                zr = psC.tile([NQ, 2, H], dt.float32, tag="yr", name=f"zr{b}{g}")
                for m in range(2):
                    nc.tensor.matmul(
                        zr[:, m, :], lhsT=h1[:, m * NQ : (m + 1) * NQ], rhs=w2[:, b, 0, :],
                        start=True, stop=True,
                    )
                zs0 = hpool.tile([NQ, H], dt.bfloat16, tag="ys0", name=f"zs0{b}{g}")
                nc.scalar.copy(zs0[:], zr[:, 0, :])
                zs1 = hpool.tile([NQ, H], dt.bfloat16, tag="ys1", name=f"zs1{b}{g}")
                nc.scalar.copy(zs1[:], zr[:, 1, :])

                gp = psD.tile([128, N], dt.float32, tag="hD", name=f"gp{b}{g}")
                nc.tensor.matmul(gp[:], lhsT=w2[:, b, 1, :], rhs=h1[:], start=True, stop=False)
                nc.tensor.matmul(gp[:], lhsT=zs0[:], rhs=A[:, 0, :], start=False, stop=False)
                nc.tensor.matmul(gp[:], lhsT=zs1[:], rhs=A[:, 1, :], start=False, stop=True)
                g1 = hpool.tile([128, N], dt.bfloat16, tag="g1", name=f"g1{b}{g}")
                nc.scalar.activation(
                    g1[:], gp[:], AF.Relu, bias=brl[:, b, 1, :], scale=1.0,
                    accum_out=pooled[b][1][:, g : g + 1],
                )

            # ---- main loop ----
            for _rep in range(repeat):
                for g in range(gpc):
                    for b in range(2):
                        xta = xpool.tile([128, N], dt.bfloat16, tag="xta", name=f"xta{b}{g}")
                        nc.sync.dma_start(out=xta[:], in_=xt_d[b, g, 0:128, :])
                        xtb = xpool.tile([F - 128, N], dt.bfloat16, tag="xtb", name=f"xtb{b}{g}")
                        nc.sync.dma_start(out=xtb[:], in_=xt_d[b, g, 128:F, :])
                        A = apool.tile([NQ, 2, N], dt.bfloat16, tag="A", name=f"A{b}{g}")
                        nc.sync.dma_start(out=A[:], in_=an_d[b, g])
                        conv_layers(b, g, A, xta, xtb)

            # ---- MLP head (f32) ----
            z1p = psD.tile([gpc, H], dt.float32, tag="hD")
            order = [pooled[0][0], pooled[0][1], pooled[1][0], pooled[1][1]]
            for k in range(4):
                nc.tensor.matmul(z1p[:], lhsT=order[k][:], rhs=l1w[:, k, :], start=(k == 0), stop=False)
            nc.tensor.matmul(z1p[:], lhsT=onesrow_f[:, 0:gpc], rhs=l1b[:], start=False, stop=True)
            z1s = rpool.tile([gpc, H], dt.float32, tag="z1s")
            nc.scalar.activation(z1s[:], z1p[:], AF.Relu, bias=0.0, scale=1.0)

            z1tp = psD.tile([H, gpc], dt.float32, tag="hD")
            nc.tensor.transpose(out=z1tp[:], in_=z1s[:], identity=ident[0:gpc, 0:gpc])
            z1t = rpool.tile([H, gpc], dt.float32, tag="z1t")
            nc.vector.tensor_copy(out=z1t[:], in_=z1tp[:])

            z2p = psD.tile([gpc, H // 2], dt.float32, tag="hD")
            nc.tensor.matmul(z2p[:], lhsT=z1t[:], rhs=l2w[:], start=True, stop=False)
            nc.tensor.matmul(z2p[:], lhsT=onesrow_f[:, 0:gpc], rhs=l2b[:], start=False, stop=True)
            z2s = rpool.tile([gpc, H // 2], dt.float32, tag="z2s")
            nc.scalar.activation(z2s[:], z2p[:], AF.Relu, bias=0.0, scale=1.0)

            z2tp = psD.tile([H // 2, gpc], dt.float32, tag="hD")
            nc.tensor.transpose(out=z2tp[:], in_=z2s[:], identity=ident[0:gpc, 0:gpc])
            z2t = rpool.tile([H // 2, gpc], dt.float32, tag="z2t")
            nc.vector.tensor_copy(out=z2t[:], in_=z2tp[:])

            z3p = psD.tile([gpc, C], dt.float32, tag="hD")
            nc.tensor.matmul(z3p[:], lhsT=z2t[:], rhs=l3w[:], start=True, stop=False)
            nc.tensor.matmul(z3p[:], lhsT=onesrow_f[:, 0:gpc], rhs=l3b[:], start=False, stop=True)

            m = rpool.tile([gpc, 1], dt.float32, tag="lsm")
            nc.vector.tensor_reduce(out=m[:], in_=z3p[:], axis=mybir.AxisListType.X, op=OP.max)
            negm = rpool.tile([gpc, 1], dt.float32, tag="lsnm")
            nc.vector.tensor_scalar(negm[:], m[:], -1.0, None, OP.mult)
            esc = rpool.tile([gpc, C], dt.float32, tag="lse")
            sume = rpool.tile([gpc, 1], dt.float32, tag="lssum")
            nc.scalar.activation(esc[:], z3p[:], AF.Exp, bias=negm[:], scale=1.0, accum_out=sume[:])
            lse = rpool.tile([gpc, 1], dt.float32, tag="lsl")
            nc.scalar.activation(lse[:], sume[:], AF.Ln, bias=0.0, scale=1.0)
            outv = rpool.tile([gpc, C], dt.float32, tag="outv")
            nc.vector.tensor_scalar(outv[:], z3p[:], negm[:], lse[:], OP.add, OP.subtract)
            nc.sync.dma_start(out=out_d[:], in_=outv[:])

    nc.compile()
    return nc


def _prep_inputs(sc_x, fc_x, sc_edge_index, fc_edge_index,
                 sc1_wrel, sc1_brel, sc1_wroot, sc2_wrel, sc2_brel, sc2_wroot,
                 fc1_wrel, fc1_brel, fc1_wroot, fc2_wrel, fc2_brel, fc2_wroot,
                 lin1_w, lin1_b, lin2_w, lin2_b, lin3_w, lin3_b, batch=None):
    import ml_dtypes

    bf = ml_dtypes.bfloat16

    def prep_x(x):
        # [B*N, F] -> [B, F, N] bf16
        return np.ascontiguousarray(np.transpose(np.asarray(x, np.float32).reshape(B, N, F), (0, 2, 1))).astype(bf)

    xt = np.stack([prep_x(sc_x), prep_x(fc_x)])  # [2, B, F, N]

    def prep_A(ei):
        # dense normalized adjacency: An[g, src, dst] = count / max(deg_dst, 1)
        ei = np.asarray(ei).astype(np.int64)
        gid = np.arange(B * EPG, dtype=np.int64) // EPG
        src = ei[0] - gid * N
        dst = ei[1] - gid * N
        flat = (gid * N + src) * N + dst
        cnt = np.bincount(flat, minlength=B * N * N).astype(np.float32).reshape(B, N, N)
        deg = cnt.sum(axis=1)  # in-degree per dst
        An = cnt / np.maximum(deg, 1.0)[:, None, :]
        # [B, N(src), N(dst)] -> [B, 100, 2, 200] (src halves interleaved as dim 2)
        An = An.reshape(B, 2, NQ, N).transpose(0, 2, 1, 3)
        return np.ascontiguousarray(An).astype(bf)

    an = np.stack([prep_A(sc_edge_index), prep_A(fc_edge_index)])  # [2, B, 100, 2, 200]

    wa = np.stack([
        np.stack([np.asarray(sc1_wrel)[:128], np.asarray(sc1_wroot)[:128]]),
        np.stack([np.asarray(fc1_wrel)[:128], np.asarray(fc1_wroot)[:128]]),
    ]).astype(bf)
    wb = np.stack([
        np.stack([np.asarray(sc1_wrel)[128:], np.asarray(sc1_wroot)[128:]]),
        np.stack([np.asarray(fc1_wrel)[128:], np.asarray(fc1_wroot)[128:]]),
    ]).astype(bf)
    w2 = np.stack([
        np.stack([np.asarray(sc2_wrel), np.asarray(sc2_wroot)]),
        np.stack([np.asarray(fc2_wrel), np.asarray(fc2_wroot)]),
    ]).astype(bf)
    brel = np.stack([
        np.stack([np.asarray(sc1_brel), np.asarray(sc2_brel)]),
        np.stack([np.asarray(fc1_brel), np.asarray(fc2_brel)]),
    ]).astype(np.float32)[:, :, :, None]

    l1w = np.asarray(lin1_w, np.float32).copy()
    l1w[:256] *= 1.0 / N  # fold mean-pool divisor for the SC branch readouts
    l1w = l1w.reshape(4, 128, H)

    return dict(
        xt=xt, an=an, wa=wa, wb=wb, w2=w2, brel=brel,
        l1w=l1w, l1b=_f32(lin1_b)[None, :], l2w=_f32(lin2_w), l2b=_f32(lin2_b)[None, :],
        l3w=_f32(lin3_w), l3b=_f32(lin3_b)[None, :],
    )


def _make_in_maps(full):
    in_maps = []
    for c in range(NCORES):
        gs = slice(c * GPC, (c + 1) * GPC)
        m = dict(full)
        m["xt"] = np.ascontiguousarray(full["xt"][:, gs])
        m["an"] = np.ascontiguousarray(full["an"][:, gs])
        in_maps.append(m)
    return in_maps


def kernel(**inputs):
    from concourse import bass_utils

    if "nc" not in _CACHE:
        _CACHE["nc"] = _build()
    nc = _CACHE["nc"]

    full = _prep_inputs(**inputs)
    in_maps = _make_in_maps(full)
    res = bass_utils.run_bass_kernel_spmd(nc, in_maps, list(range(NCORES)))
    return np.concatenate([res.results[i]["out"] for i in range(NCORES)], axis=0).astype(np.float32)


# revision 6
# speedup vs baseline: 1.6240x; 1.6240x over previous
"""Trainium2 Bass kernel for the ASAP dual-branch GNN (GraphConv mean-aggr).

Strategy (data-parallel over graphs, 32 graphs per NeuronCore):
  * Host folds each graph's edge list into a dense normalized adjacency
    An[src, dst] = count(src->dst) / max(deg_dst, 1) in bf16 (one bincount
    over all edges), so on-device the GraphConv layers are pure dense
    matmuls: h = relu( wrel^T (x An) + wroot^T x + brel ).
  * Graphs processed in pairs packed side-by-side in the free axis, so the
    shared-weight root matmuls stream 400 columns at once and DMAs move
    800B+ contiguous rows.
  * An split into two 100-row src halves serving as the PE moving operand
    with the node-major ys = x@wrel halves as stationary weights.
  * Engine balance: PE matmuls; ACT pair-wide relu+bias; psum->bf16 copies
    on GpSimd+DVE; per-graph pooled readout via DVE free-axis tensor_reduce.
  * Small MLP head + log_softmax computed per-core in f32; no collectives.
Host side only does sharding/layout: adjacency histogram + degree fold,
transposes, dtype casts, and constant folding of the mean-pool 1/200 into
lin1_w.
"""

import os
import sys

import numpy as np

if "/opt/trn_rl_repo" not in sys.path:
    sys.path.insert(0, "/opt/trn_rl_repo")

B, N, EPG = 256, 200, 3200
F, H, C = 200, 128, 2
NCORES = 8
GPC = B // NCORES  # graphs per core
NPAIR = GPC // 2
NQ = 100  # src-half width

_CACHE = {}


def _f32(x):
    return np.ascontiguousarray(x, dtype=np.float32)


def _bf16(x):
    import ml_dtypes

    return np.ascontiguousarray(np.asarray(x, dtype=np.float32).astype(ml_dtypes.bfloat16))


def _build(gpc=GPC, repeat=1):
    import concourse.bass as bass
    import concourse.tile as tile
    from concourse import bacc, mybir

    dt = mybir.dt
    AF = mybir.ActivationFunctionType
    OP = mybir.AluOpType
    assert gpc % 2 == 0
    npair = gpc // 2

    nc = bacc.Bacc("TRN2", target_bir_lowering=False, debug=False)

    # ---- DRAM I/O (graph pairs packed in the free axis) ----
    xt_d = nc.dram_tensor("xt", [2, npair, F, 2, N], dt.bfloat16, kind="ExternalInput").ap()
    an_d = nc.dram_tensor("an", [2, npair, NQ, 2, 2, N], dt.bfloat16, kind="ExternalInput").ap()
    wa_d = nc.dram_tensor("wa", [2, 2, 128, H], dt.bfloat16, kind="ExternalInput").ap()
    wb_d = nc.dram_tensor("wb", [2, 2, F - 128, H], dt.bfloat16, kind="ExternalInput").ap()
    w2_d = nc.dram_tensor("w2", [2, 2, H, H], dt.bfloat16, kind="ExternalInput").ap()
    br_d = nc.dram_tensor("brel", [2, 2, H, 1], dt.float32, kind="ExternalInput").ap()
    l1w_d = nc.dram_tensor("l1w", [4, H, H], dt.float32, kind="ExternalInput").ap()
    l1b_d = nc.dram_tensor("l1b", [1, H], dt.float32, kind="ExternalInput").ap()
    l2w_d = nc.dram_tensor("l2w", [H, H // 2], dt.float32, kind="ExternalInput").ap()
    l2b_d = nc.dram_tensor("l2b", [1, H // 2], dt.float32, kind="ExternalInput").ap()
    l3w_d = nc.dram_tensor("l3w", [H // 2, C], dt.float32, kind="ExternalInput").ap()
    l3b_d = nc.dram_tensor("l3b", [1, C], dt.float32, kind="ExternalInput").ap()
    out_d = nc.dram_tensor("out", [gpc, C], dt.float32, kind="ExternalOutput").ap()

    # ---- inline constants ----
    onesrow_f_d = nc.inline_tensor(_f32(np.ones((1, 128), np.float32)), "onesrowf").ap()
    ident_d = nc.inline_tensor(_f32(np.eye(128, dtype=np.float32)), "identf").ap()

    with tile.TileContext(nc) as tc:
        with (
            tc.tile_pool(name="cpool", bufs=1) as cpool,
            tc.tile_pool(name="xpool", bufs=4) as xpool,
            tc.tile_pool(name="apool", bufs=4) as apool,
            tc.tile_pool(name="hpool", bufs=4) as hpool,
            tc.tile_pool(name="spool", bufs=8) as spool,
            tc.tile_pool(name="rpool", bufs=4) as rpool,
            tc.tile_pool(name="psC", bufs=4, space="PSUM") as psC,
            tc.tile_pool(name="psD", bufs=3, space="PSUM") as psD,
        ):
            # ---- load constants / weights ----
            onesrow_f = cpool.tile([1, 128], dt.float32)
            nc.sync.dma_start(out=onesrow_f[:], in_=onesrow_f_d[:])
            ident = cpool.tile([128, 128], dt.float32)
            nc.sync.dma_start(out=ident[:], in_=ident_d[:])

            wa = cpool.tile([128, 2, 2, H], dt.bfloat16)
            wb = cpool.tile([F - 128, 2, 2, H], dt.bfloat16)
            w2 = cpool.tile([128, 2, 2, H], dt.bfloat16)
            brl = cpool.tile([128, 2, 2, 1], dt.float32)
            for b in range(2):
                for k in range(2):
                    nc.sync.dma_start(out=wa[:, b, k, :], in_=wa_d[b, k])
                    nc.sync.dma_start(out=wb[:, b, k, :], in_=wb_d[b, k])
                    nc.sync.dma_start(out=w2[:, b, k, :], in_=w2_d[b, k])
                    nc.sync.dma_start(out=brl[:, b, k, :], in_=br_d[b, k])
            l1w = cpool.tile([128, 4, H], dt.float32)
            for k in range(4):
                nc.sync.dma_start(out=l1w[:, k, :], in_=l1w_d[k])
            l1b = cpool.tile([1, H], dt.float32)
            nc.sync.dma_start(out=l1b[:], in_=l1b_d[:])
            l2w = cpool.tile([H, H // 2], dt.float32)
            nc.sync.dma_start(out=l2w[:], in_=l2w_d[:])
            l2b = cpool.tile([1, H // 2], dt.float32)
            nc.sync.dma_start(out=l2b[:], in_=l2b_d[:])
            l3w = cpool.tile([H // 2, C], dt.float32)
            nc.sync.dma_start(out=l3w[:], in_=l3w_d[:])
            l3b = cpool.tile([1, C], dt.float32)
            nc.sync.dma_start(out=l3b[:], in_=l3b_d[:])

            pooled = [
                [cpool.tile([128, gpc], dt.float32, tag=f"pool{b}{l}", name=f"pooled{b}{l}") for l in range(2)]
                for b in range(2)
            ]

            def conv_pair(b, p, A, xta, xtb):
                # Layer 1: ys = x @ wrel (node-major halves), per graph
                yss = []
                for g in range(2):
                    yr = psC.tile([NQ, 2, H], dt.float32, tag="yr", name=f"yr{b}{p}{g}")
                    for m in range(2):
                        nc.tensor.matmul(
                            yr[:, m, :], lhsT=xta[:, g, m * NQ : (m + 1) * NQ], rhs=wa[:, b, 0, :],
                            start=True, stop=False,
                        )
                        nc.tensor.matmul(
                            yr[:, m, :], lhsT=xtb[:, g, m * NQ : (m + 1) * NQ], rhs=wb[:, b, 0, :],
                            start=False, stop=True,
                        )
                    ys = spool.tile([NQ, 2, H], dt.bfloat16, tag="ys", name=f"ys{b}{p}{g}")
                    if g == 0:
                        nc.vector.tensor_copy(out=ys[:], in_=yr[:])
                    else:
                        nc.scalar.copy(out=ys[:], in_=yr[:])
                    yss.append(ys)

                hp = psD.tile([128, 2, N], dt.float32, tag="hD", name=f"hp{b}{p}")
                nc.tensor.matmul(hp[:, :, :], lhsT=wa[:, b, 1, :], rhs=xta[:, :, :], start=True, stop=False)
                nc.tensor.matmul(hp[:, :, :], lhsT=wb[:, b, 1, :], rhs=xtb[:, :, :], start=False, stop=False)
                for g in range(2):
                    nc.tensor.matmul(hp[:, g, :], lhsT=yss[g][:, 0, :], rhs=A[:, g, 0, :], start=False, stop=False)
                    nc.tensor.matmul(
                        hp[:, g, :], lhsT=yss[g][:, 1, :], rhs=A[:, g, 1, :],
                        start=False, stop=(g == 1),
                    )
                h1 = hpool.tile([128, 2, N], dt.bfloat16, tag="h1", name=f"h1{b}{p}")
                nc.scalar.activation(h1[:, :, :], hp[:, :, :], AF.Relu, bias=brl[:, b, 0, :], scale=1.0)
                nc.vector.tensor_reduce(
                    out=pooled[b][0][:, 2 * p : 2 * p + 2], in_=h1[:, :, :],
                    axis=mybir.AxisListType.X, op=OP.add,
                )

                # Layer 2
                zss = []
                for g in range(2):
                    zr = psC.tile([NQ, 2, H], dt.float32, tag="yr", name=f"zr{b}{p}{g}")
                    for m in range(2):
                        nc.tensor.matmul(
                            zr[:, m, :], lhsT=h1[:, g, m * NQ : (m + 1) * NQ], rhs=w2[:, b, 0, :],
                            start=True, stop=True,
                        )
                    zs = spool.tile([NQ, 2, H], dt.bfloat16, tag="ys", name=f"zs{b}{p}{g}")
                    if g == 0:
                        nc.vector.tensor_copy(out=zs[:], in_=zr[:])
                    else:
                        nc.scalar.copy(out=zs[:], in_=zr[:])
                    zss.append(zs)

                gp = psD.tile([128, 2, N], dt.float32, tag="hD", name=f"gp{b}{p}")
                nc.tensor.matmul(gp[:, :, :], lhsT=w2[:, b, 1, :], rhs=h1[:, :, :], start=True, stop=False)
                for g in range(2):
                    nc.tensor.matmul(gp[:, g, :], lhsT=zss[g][:, 0, :], rhs=A[:, g, 0, :], start=False, stop=False)
                    nc.tensor.matmul(
                        gp[:, g, :], lhsT=zss[g][:, 1, :], rhs=A[:, g, 1, :],
                        start=False, stop=(g == 1),
                    )
                g1 = hpool.tile([128, 2, N], dt.bfloat16, tag="g1", name=f"g1{b}{p}")
                nc.scalar.activation(g1[:, :, :], gp[:, :, :], AF.Relu, bias=brl[:, b, 1, :], scale=1.0)
                nc.vector.tensor_reduce(
                    out=pooled[b][1][:, 2 * p : 2 * p + 2], in_=g1[:, :, :],
                    axis=mybir.AxisListType.X, op=OP.add,
                )

            # ---- main loop: graph pairs ----
            for _rep in range(repeat):
                for p in range(npair):
                    for b in range(2):
                        xta = xpool.tile([128, 2, N], dt.bfloat16, tag="xta", name=f"xta{b}{p}")
                        nc.sync.dma_start(out=xta[:], in_=xt_d[b, p, 0:128, :, :])
                        xtb = xpool.tile([F - 128, 2, N], dt.bfloat16, tag="xtb", name=f"xtb{b}{p}")
                        nc.sync.dma_start(out=xtb[:], in_=xt_d[b, p, 128:F, :, :])
                        A = apool.tile([NQ, 2, 2, N], dt.bfloat16, tag="A", name=f"A{b}{p}")
                        nc.sync.dma_start(out=A[:], in_=an_d[b, p])
                        conv_pair(b, p, A, xta, xtb)

            # ---- MLP head (f32) ----
            z1p = psD.tile([gpc, H], dt.float32, tag="hD")
            order = [pooled[0][0], pooled[0][1], pooled[1][0], pooled[1][1]]
            for k in range(4):
                nc.tensor.matmul(z1p[:], lhsT=order[k][:], rhs=l1w[:, k, :], start=(k == 0), stop=False)
            nc.tensor.matmul(z1p[:], lhsT=onesrow_f[:, 0:gpc], rhs=l1b[:], start=False, stop=True)
            z1s = rpool.tile([gpc, H], dt.float32, tag="z1s")
            nc.scalar.activation(z1s[:], z1p[:], AF.Relu, bias=0.0, scale=1.0)

            z1tp = psD.tile([H, gpc], dt.float32, tag="hD")
            nc.tensor.transpose(out=z1tp[:], in_=z1s[:], identity=ident[0:gpc, 0:gpc])
            z1t = rpool.tile([H, gpc], dt.float32, tag="z1t")
            nc.vector.tensor_copy(out=z1t[:], in_=z1tp[:])

            z2p = psD.tile([gpc, H // 2], dt.float32, tag="hD")
            nc.tensor.matmul(z2p[:], lhsT=z1t[:], rhs=l2w[:], start=True, stop=False)
            nc.tensor.matmul(z2p[:], lhsT=onesrow_f[:, 0:gpc], rhs=l2b[:], start=False, stop=True)
            z2s = rpool.tile([gpc, H // 2], dt.float32, tag="z2s")
            nc.scalar.activation(z2s[:], z2p[:], AF.Relu, bias=0.0, scale=1.0)

            z2tp = psD.tile([H // 2, gpc], dt.float32, tag="hD")
            nc.tensor.transpose(out=z2tp[:], in_=z2s[:], identity=ident[0:gpc, 0:gpc])
            z2t = rpool.tile([H // 2, gpc], dt.float32, tag="z2t")
            nc.vector.tensor_copy(out=z2t[:], in_=z2tp[:])

            z3p = psD.tile([gpc, C], dt.float32, tag="hD")
            nc.tensor.matmul(z3p[:], lhsT=z2t[:], rhs=l3w[:], start=True, stop=False)
            nc.tensor.matmul(z3p[:], lhsT=onesrow_f[:, 0:gpc], rhs=l3b[:], start=False, stop=True)

            m = rpool.tile([gpc, 1], dt.float32, tag="lsm")
            nc.vector.tensor_reduce(out=m[:], in_=z3p[:], axis=mybir.AxisListType.X, op=OP.max)
            negm = rpool.tile([gpc, 1], dt.float32, tag="lsnm")
            nc.vector.tensor_scalar(negm[:], m[:], -1.0, None, OP.mult)
            esc = rpool.tile([gpc, C], dt.float32, tag="lse")
            sume = rpool.tile([gpc, 1], dt.float32, tag="lssum")
            nc.scalar.activation(esc[:], z3p[:], AF.Exp, bias=negm[:], scale=1.0, accum_out=sume[:])
            lse = rpool.tile([gpc, 1], dt.float32, tag="lsl")
            nc.scalar.activation(lse[:], sume[:], AF.Ln, bias=0.0, scale=1.0)
            outv = rpool.tile([gpc, C], dt.float32, tag="outv")
            nc.vector.tensor_scalar(outv[:], z3p[:], negm[:], lse[:], OP.add, OP.subtract)
            nc.sync.dma_start(out=out_d[:], in_=outv[:])

    nc.compile()
    return nc


def _prep_inputs(sc_x, fc_x, sc_edge_index, fc_edge_index,
                 sc1_wrel, sc1_brel, sc1_wroot, sc2_wrel, sc2_brel, sc2_wroot,
                 fc1_wrel, fc1_brel, fc1_wroot, fc2_wrel, fc2_brel, fc2_wroot,
                 lin1_w, lin1_b, lin2_w, lin2_b, lin3_w, lin3_b, batch=None):
    import ml_dtypes

    bf = ml_dtypes.bfloat16

    def prep_x(x):
        # [B*N, F] -> [B/2, F, 2, N] bf16 (graph pairs packed in free axis)
        x = np.asarray(x, np.float32).reshape(B // 2, 2, N, F).transpose(0, 3, 1, 2)
        return np.ascontiguousarray(x).astype(bf)

    xt = np.stack([prep_x(sc_x), prep_x(fc_x)])  # [2, B/2, F, 2, N]

    def prep_A(ei):
        # dense normalized adjacency: An[g, src, dst] = count / max(deg_dst, 1)
        ei = np.asarray(ei).astype(np.int64)
        gid = np.arange(B * EPG, dtype=np.int64) // EPG
        src = ei[0] - gid * N
        dst = ei[1] - gid * N
        flat = (gid * N + src) * N + dst
        cnt = np.bincount(flat, minlength=B * N * N).astype(np.float32).reshape(B, N, N)
        deg = cnt.sum(axis=1)  # in-degree per dst
        An = cnt / np.maximum(deg, 1.0)[:, None, :]
        # [B, N(src), N(dst)] -> [B/2, 100, 2(graph), 2(src half), 200]
        An = An.reshape(B // 2, 2, 2, NQ, N).transpose(0, 3, 1, 2, 4)
        return np.ascontiguousarray(An).astype(bf)

    an = np.stack([prep_A(sc_edge_index), prep_A(fc_edge_index)])  # [2, B/2, 100, 2, 2, 200]

    wa = np.stack([
        np.stack([np.asarray(sc1_wrel)[:128], np.asarray(sc1_wroot)[:128]]),
        np.stack([np.asarray(fc1_wrel)[:128], np.asarray(fc1_wroot)[:128]]),
    ]).astype(bf)
    wb = np.stack([
        np.stack([np.asarray(sc1_wrel)[128:], np.asarray(sc1_wroot)[128:]]),
        np.stack([np.asarray(fc1_wrel)[128:], np.asarray(fc1_wroot)[128:]]),
    ]).astype(bf)
    w2 = np.stack([
        np.stack([np.asarray(sc2_wrel), np.asarray(sc2_wroot)]),
        np.stack([np.asarray(fc2_wrel), np.asarray(fc2_wroot)]),
    ]).astype(bf)
    brel = np.stack([
        np.stack([np.asarray(sc1_brel), np.asarray(sc2_brel)]),
        np.stack([np.asarray(fc1_brel), np.asarray(fc2_brel)]),
    ]).astype(np.float32)[:, :, :, None]

    l1w = np.asarray(lin1_w, np.float32).copy()
    l1w[:256] *= 1.0 / N  # fold mean-pool divisor for the SC branch readouts
    l1w = l1w.reshape(4, 128, H)

    return dict(
        xt=xt, an=an, wa=wa, wb=wb, w2=w2, brel=brel,
        l1w=l1w, l1b=_f32(lin1_b)[None, :], l2w=_f32(lin2_w), l2b=_f32(lin2_b)[None, :],
        l3w=_f32(lin3_w), l3b=_f32(lin3_b)[None, :],
    )


def _make_in_maps(full):
    in_maps = []
    for c in range(NCORES):
        ps = slice(c * NPAIR, (c + 1) * NPAIR)
        m = dict(full)
        m["xt"] = np.ascontiguousarray(full["xt"][:, ps])
        m["an"] = np.ascontiguousarray(full["an"][:, ps])
        in_maps.append(m)
    return in_maps


def kernel(**inputs):
    from concourse import bass_utils

    if "nc" not in _CACHE:
        _CACHE["nc"] = _build()
    nc = _CACHE["nc"]

    full = _prep_inputs(**inputs)
    in_maps = _make_in_maps(full)
    res = bass_utils.run_bass_kernel_spmd(nc, in_maps, list(range(NCORES)))
    return np.concatenate([res.results[i]["out"] for i in range(NCORES)], axis=0).astype(np.float32)
